# revision 12
# baseline (speedup 1.0000x reference)
"""Causal self-attention Trainium2 Bass kernel.

Problem: B=4, T=2048, C=1024, H=16 heads, head_dim=64, fp32.
    qkv = x @ Wqkv + bqkv ; per-head causal softmax attention ; out = attn @ Wo + bo

Sharding (8 NeuronCores): core c -> (batch b = c//2, head-group g = c%2).
Each core computes qkv for its batch restricted to its 8 heads, attention for
those heads, and a partial output projection against its 512 rows of Wo.
The host sums the two partials of each batch pair (the tensor-parallel
all-reduce), adds bo, and stacks batches.

On-core dataflow (matmul dtype MM_DT; PSUM accumulation is always fp32):
  Phase 1:  qT,kT [512hd x 2048t] and v [2048t x 512hd] from xT (host-side
            transpose of x[b]) and Wq/Wk/Wv column slices. bq/bk applied as
            per-partition adds during the PSUM->SBUF copy; bv as a K=1
            rank-1 matmul update.
  Phase 2:  per (head, 512-query block): S_T[k,q] = kT-tile^T @ qT, exp via
            ACT (scale=1/8 folded in; scores bounded ~|3.2| so no max
            subtraction), causal masking via a host triangular tile +
            sub-range accumulation, attnT_aug = [v|1]^T @ expS_T accumulated
            over key tiles (row 64 = softmax denominator).  Normalization is
            software-pipelined one block behind: denom row -> SBUF, ones x
            denom broadcast matmul -> PSUM, 64-lane reciprocal -> SBUF,
            multiply into attnT.
  Phase 3:  out_partial[t,c] = attnT-tile^T @ Wo-rows; bo added on host.
"""

import sys

if "/opt/trn_rl_repo" not in sys.path:
    sys.path.insert(0, "/opt/trn_rl_repo")

import numpy as np

import concourse.bass as bass
import concourse.tile as tile
from concourse import bacc, mybir
from concourse.bass_utils import run_bass_kernel_spmd

F32 = mybir.dt.float32
F32R = mybir.dt.float32r
F16 = mybir.dt.float16
BF16 = mybir.dt.bfloat16
EXP = mybir.ActivationFunctionType.Exp

# Matmul operand dtype: F32R (2 PE cycles/row, ~1.5e-4 matmul relerr) or
# F16/BF16 (1 cycle/row).
import os as _os

MM_DT = {"f32r": F32R, "f16": F16, "bf16": BF16}[_os.environ.get("MM_DT", "f32r")]

B, T, C = 4, 2048, 1024
H, D = 16, 64
HPC = 8          # heads per core
HD = HPC * D     # 512: per-core head-dim slab
N_CORES = 8
SCALE = D ** -0.5


def _np_of(dt):
    return np.dtype(mybir.dt.np(dt))


def build_nc(mm_dt=None):
    mm_dt = mm_dt or MM_DT
    nc = bacc.Bacc("TRN2", target_bir_lowering=False, debug=False)

    xT = nc.dram_tensor("xT", [C, T], mm_dt, kind="ExternalInput")
    wq = nc.dram_tensor("wq", [C, HD], mm_dt, kind="ExternalInput")
    wk = nc.dram_tensor("wk", [C, HD], mm_dt, kind="ExternalInput")
    wv = nc.dram_tensor("wv", [C, HD], mm_dt, kind="ExternalInput")
    wo = nc.dram_tensor("wo", [HD, C], mm_dt, kind="ExternalInput")
    # bq/bk as [128, HD//128] columns (per-partition adds in qkvT layout)
    bqc = nc.dram_tensor("bqc", [128, HD // 128], F32, kind="ExternalInput")
    bkc = nc.dram_tensor("bkc", [128, HD // 128], F32, kind="ExternalInput")
    bv = nc.dram_tensor("bv", [1, HD], mm_dt, kind="ExternalInput")
    tri = nc.dram_tensor("tri", [128, 128], mm_dt, kind="ExternalInput")
    out = nc.dram_tensor("out", [T, C], F32, kind="ExternalOutput")

    KO = C // 128        # 8 contraction tiles over C
    TC = T // 512        # 4 t-chunks of 512
    NQ = T // 512        # 4 query blocks per head
    NKT = T // 128       # 16 key tiles
    HDO = HD // 128      # 4 hd tiles

    with tile.TileContext(nc) as tc:
        const = tc.alloc_tile_pool(name="const", bufs=1)
        persist = tc.alloc_tile_pool(name="persist", bufs=1)
        # PSUM banks: mm [128,1024]=2 banks x2 bufs + aug x2 + bc x1 = 7 of 8
        psum = tc.alloc_tile_pool(name="psum", bufs=2, space="PSUM")
        psum_aug = tc.alloc_tile_pool(name="psum_aug", bufs=2, space="PSUM")
        psum_bc = tc.alloc_tile_pool(name="psum_bc", bufs=1, space="PSUM")

        # --- constants ---
        ones_f = const.tile([1, 512], F32)
        ones_r = const.tile([1, 512], mm_dt)
        nc.vector.memset(ones_f[:], 1.0)
        nc.vector.tensor_copy(ones_r[:], ones_f[:])
        ones_col_f = const.tile([128, 1], F32)
        nc.vector.memset(ones_col_f[:], 1.0)
        tri_sb = const.tile([128, 128], mm_dt)
        nc.sync.dma_start(tri_sb[:], tri[:, :])
        bqc_sb = const.tile([128, HD // 128], F32)
        bkc_sb = const.tile([128, HD // 128], F32)
        bv_sb = const.tile([1, HD], mm_dt)
        nc.sync.dma_start(bqc_sb[:], bqc[:, :])
        nc.sync.dma_start(bkc_sb[:], bkc[:, :])
        nc.sync.dma_start(bv_sb[:], bv[:, :])

        # --- persistent tensors ---
        qT_sb = persist.tile([128, HDO, T], mm_dt)   # [colpart, hd-outer, t]
        kT_sb = persist.tile([128, HDO, T], mm_dt)
        v_sb = persist.tile([128, NKT, HPC, D + 1], mm_dt)  # [tpart, ktile, head, d|1]
        nc.vector.tensor_copy(
            v_sb[:, :, :, D], ones_col_f[:, 0:1].to_broadcast([128, NKT, HPC])
        )

        # ---------------- Phase 1: qT, kT, v ----------------
        ph1 = tc.alloc_tile_pool(name="ph1", bufs=1)
        wq_sb = ph1.tile([128, KO, HD], mm_dt)
        wk_sb = ph1.tile([128, KO, HD], mm_dt)
        wv_sb = ph1.tile([128, KO, HD], mm_dt)
        for w_sb, w_d in ((wq_sb, wq), (wk_sb, wk), (wv_sb, wv)):
            for ko in range(KO):
                nc.sync.dma_start(w_sb[:, ko], w_d[ko * 128 : (ko + 1) * 128, :])
        xt_pool = tc.alloc_tile_pool(name="xt", bufs=9)

        for tc4 in range(TC):
            ts_ = slice(tc4 * 512, (tc4 + 1) * 512)
            xt = []
            for ko in range(KO):
                t_ = xt_pool.tile([128, 512], mm_dt, tag="xt")
                nc.sync.dma_start(t_[:], xT[ko * 128 : (ko + 1) * 128, ts_])
                xt.append(t_)
            # qT / kT column tiles: psum[colpart, t]; bias as per-partition add
            for w_sb, b_sb, dst in ((wq_sb, bqc_sb, qT_sb), (wk_sb, bkc_sb, kT_sb)):
                for i in range(HDO):
                    cs = slice(i * 128, (i + 1) * 128)
                    ps = psum.tile([128, 512], F32, tag="mm")
                    for ko in range(KO):
                        nc.tensor.matmul(
                            ps[:], w_sb[:, ko, cs], xt[ko][:],
                            start=(ko == 0), stop=(ko == KO - 1),
                        )
                    nc.vector.tensor_scalar_add(
                        dst[:, i, ts_], ps[:], b_sb[:, i : i + 1]
                    )
            # v tiles: psum[tpart, hd]; bias via K=1 rank-1 matmul
            for s in range(4):
                kt = tc4 * 4 + s
                ps = psum.tile([128, 512], F32, tag="mm")
                for ko in range(KO):
                    nc.tensor.matmul(
                        ps[:], xt[ko][:, s * 128 : (s + 1) * 128], wv_sb[:, ko, :],
                        start=(ko == 0), stop=False,
                    )
                nc.tensor.matmul(
                    ps[:], ones_r[0:1, 0:128], bv_sb[0:1, :],
                    start=False, stop=True, skip_group_check=True,
                )
                nc.scalar.copy(
                    v_sb[:, kt, :, 0:D], ps[:].rearrange("p (h d) -> p h d", h=HPC)
                )

        xt_pool.release()
        ph1.release()

        # -------- Phase 2: attention, with phase-3 rounds interleaved --------
        # Per head-pair hp (= hd tile co), after its attnT columns are done,
        # run the output-projection matmuls for that hd tile as
        # ACT-independent PE filler (lets the PE clock re-warm), accumulating
        # into an SBUF fp32 buffer.
        ph2 = tc.alloc_tile_pool(name="ph2", bufs=1)
        attnT_sb = ph2.tile([128, HDO, T], mm_dt)
        out_acc = ph2.tile([128, NKT, C], F32)
        wo_sb = ph2.tile([128, HDO, C], mm_dt)
        for ko in range(HDO):
            nc.sync.dma_start(wo_sb[:, ko], wo[ko * 128 : (ko + 1) * 128, :])
        e_pool = tc.alloc_tile_pool(name="e", bufs=6)
        r_pool = tc.alloc_tile_pool(name="r", bufs=3)

        pending = None  # (aug, drow, pr, co, q) awaiting normalization

        def flush_norm():
            nonlocal pending
            if pending is None:
                return
            aug, drow, pr, co, q = pending
            bc = psum_bc.tile([64, 512], F32, tag="bc")
            nc.tensor.matmul(bc[:], ones_r[0:1, 0:64], drow[:], start=True, stop=True)
            rec = r_pool.tile([64, 512], F32, tag="rec")
            # ~4e-6 relerr, ~5x faster than exact reciprocal; denom >= ~0.04
            nc.vector.reciprocal_approx_fast(rec[:], bc[:])
            nc.vector.tensor_mul(
                attnT_sb[pr : pr + 64, co, q * 512 : (q + 1) * 512],
                aug[0:D, :], rec[:],
            )
            pending = None

        for h in range(HPC):
            co, pr = h // 2, (h % 2) * 64
            for q in range(NQ):
                jmax = 4 * q + 3
                aug = psum_aug.tile([D + 1, 512], F32, tag="aug")
                j = 0
                while j <= jmax:
                    if j + 1 < 4 * q:
                        # two full key tiles: one 1024-wide exp
                        ps = psum.tile([128, 1024], F32, tag="mm")
                        e = e_pool.tile([128, 1024], mm_dt, tag="e")
                        for u in range(2):
                            nc.tensor.matmul(
                                ps[:, u * 512 : (u + 1) * 512],
                                kT_sb[pr : pr + 64, co, (j + u) * 128 : (j + u + 1) * 128],
                                qT_sb[pr : pr + 64, co, q * 512 : (q + 1) * 512],
                                start=True, stop=True, skip_group_check=True,
                            )
                        nc.scalar.activation(e[:], ps[:], EXP, scale=SCALE)
                        for u in range(2):
                            nc.tensor.matmul(
                                aug[:], v_sb[:, j + u, h, :],
                                e[:, u * 512 : (u + 1) * 512],
                                start=(j + u == 0), stop=False,
                                skip_group_check=True,
                            )
                        j += 2
                        continue
                    diag = j >= 4 * q
                    c0 = 128 * (j - 4 * q) if diag else 0
                    ncol = 512 - c0
                    ps = psum.tile([128, 1024], F32, tag="mm")
                    nc.tensor.matmul(
                        ps[:, :ncol],
                        kT_sb[pr : pr + 64, co, j * 128 : (j + 1) * 128],
                        qT_sb[pr : pr + 64, co, q * 512 + c0 : (q + 1) * 512],
                        start=True, stop=True,
                    )
                    e = e_pool.tile([128, 1024], mm_dt, tag="e")
                    nc.scalar.activation(e[:, :ncol], ps[:, :ncol], EXP, scale=SCALE)
                    if diag:
                        nc.vector.tensor_mul(e[:, 0:128], e[:, 0:128], tri_sb[:])
                    nc.tensor.matmul(
                        aug[:, c0:], v_sb[:, j, h, :], e[:, :ncol],
                        start=(j == 0), stop=(j == jmax), skip_group_check=True,
                    )
                    j += 1
                # denominator row out of PSUM, then normalize previous block
                drow = r_pool.tile([1, 512], mm_dt, tag="drow")
                with nc.allow_low_precision(reason="softmax denom rounding"):
                    nc.vector.tensor_copy(drow[:], aug[D : D + 1, :])
                flush_norm()
                pending = (aug, drow, pr, co, q)
            # after the second head of a pair: output-projection round for
            # this hd tile (PE filler independent of ACT)
            if h % 2 == 1:
                flush_norm()
                ko = co
                for tt in range(NKT):
                    ps = psum.tile([128, 1024], F32, tag="mm")
                    for cc in range(2):
                        nc.tensor.matmul(
                            ps[:, cc * 512 : (cc + 1) * 512],
                            attnT_sb[:, ko, tt * 128 : (tt + 1) * 128],
                            wo_sb[:, ko, cc * 512 : (cc + 1) * 512],
                            start=True, stop=True, skip_group_check=True,
                        )
                    if ko == 0:
                        nc.vector.tensor_copy(out_acc[:, tt, :], ps[:])
                    else:
                        nc.vector.tensor_add(out_acc[:, tt, :], out_acc[:, tt, :], ps[:])

        for tt in range(NKT):
            nc.sync.dma_start(out[tt * 128 : (tt + 1) * 128, :], out_acc[:, tt, :])

        r_pool.release()
        e_pool.release()
        ph2.release()
        psum_bc.release()
        psum_aug.release()
        psum.release()
        persist.release()
        const.release()

    nc.finalize()
    return nc


_NC_CACHE = {}


def _get_nc(mm_dt=None):
    key = str(mm_dt or MM_DT)
    if key not in _NC_CACHE:
        _NC_CACHE[key] = build_nc(mm_dt)
    return _NC_CACHE[key]


def make_in_maps(x, Wqkv, bqkv, Wo, mm_dt=None):
    mdt = _np_of(mm_dt or MM_DT)
    x = np.asarray(x, dtype=np.float32)
    Wqkv = np.asarray(Wqkv, dtype=np.float32)
    bqkv = np.asarray(bqkv, dtype=np.float32)
    Wo = np.asarray(Wo, dtype=np.float32)

    w3 = Wqkv.reshape(C, 3, H, D)
    b3 = bqkv.reshape(3, H, D)
    wo4 = Wo.reshape(H, D, C)
    tri = np.triu(np.ones((128, 128), dtype=np.float32))

    in_maps = []
    for c in range(N_CORES):
        b, g = c // 2, c % 2
        hs = slice(g * HPC, (g + 1) * HPC)
        bq = b3[0, hs].reshape(HD)
        bk = b3[1, hs].reshape(HD)
        in_maps.append({
            "xT": np.ascontiguousarray(x[b].T).astype(mdt),
            "wq": np.ascontiguousarray(w3[:, 0, hs, :].reshape(C, HD)).astype(mdt),
            "wk": np.ascontiguousarray(w3[:, 1, hs, :].reshape(C, HD)).astype(mdt),
            "wv": np.ascontiguousarray(w3[:, 2, hs, :].reshape(C, HD)).astype(mdt),
            "wo": np.ascontiguousarray(wo4[hs].reshape(HD, C)).astype(mdt),
            "bqc": np.ascontiguousarray(bq.reshape(HD // 128, 128).T).astype(np.float32),
            "bkc": np.ascontiguousarray(bk.reshape(HD // 128, 128).T).astype(np.float32),
            "bv": b3[2, hs].reshape(1, HD).astype(mdt),
            "tri": tri.astype(mdt),
        })
    return in_maps


def run(x, Wqkv, bqkv, Wo, bo, mm_dt=None, **spmd_kwargs):
    nc = _get_nc(mm_dt)
    in_maps = make_in_maps(x, Wqkv, bqkv, Wo, mm_dt=mm_dt)
    res = run_bass_kernel_spmd(nc, in_maps, core_ids=list(range(N_CORES)),
                               **spmd_kwargs)
    bo = np.asarray(bo, dtype=np.float32)
    out = np.empty((B, T, C), dtype=np.float32)
    for b in range(B):
        out[b] = res.results[2 * b]["out"] + res.results[2 * b + 1]["out"] + bo
    return out, res


def kernel(x, Wqkv, bqkv, Wo, bo):
    out, _ = run(x, Wqkv, bqkv, Wo, bo)
    return out


# revision 14
# speedup vs baseline: 1.2967x; 1.2967x over previous
"""Causal self-attention Trainium2 Bass kernel.

Problem: B=4, T=2048, C=1024, H=16 heads, head_dim=64, fp32.
    qkv = x @ Wqkv + bqkv ; per-head causal softmax attention ; out = attn @ Wo + bo

Sharding (8 NeuronCores): core c -> (batch b = c//2, head-group g = c%2).
Each core computes qkv for its batch restricted to its 8 heads, attention for
those heads, and a partial output projection against its 512 rows of Wo.
The host sums the two partials of each batch pair (the tensor-parallel
all-reduce), adds bo, and stacks batches.

On-core dataflow (matmul dtype MM_DT = fp16 by default; PSUM accumulation is
always fp32):

  The kernel is emitted QUERY-BLOCK-MAJOR so that projection (phase-1) matmul
  work interleaves with attention (phase-2) work on the PE.  Attention
  couples PE->ACT->PE (scores -> exp -> weighted sum), and the exp stream on
  the Scalar engine is slightly slower than the PE's attention work, so a
  pure attention phase starves the PE in sub-microsecond gaps; the PE clock
  monitor then halves the PE clock (K=4/8 gating needs ~3.4us of
  uninterrupted work to re-warm).  Interleaving the independent qkv
  projection matmuls keeps the PE saturated.

    round tc=0:  qT/kT/v chunk 0 (t in [0,512))
    round q:     attention blocks (h, q) for all 8 heads,
                 interleaved with qT/kT/v chunk q+1
    tail:        out_partial[t,c] = attnT-tile^T @ Wo-rows (PSUM-accumulated)

  Attention per (head, 512-query block): S_T[k,q] = kT-tile^T @ qT, exp via
  ACT (scale=1/8 folded in; scores bounded ~|3.2| so no max subtraction;
  full key-tile pairs share one 1024-wide exp), causal masking via a host
  triangular tile + sub-range accumulation, attnT_aug = [v|1]^T @ expS_T
  accumulated over key tiles (row 64 = softmax denominator).  Normalization
  is software-pipelined one block behind: denom row -> SBUF, ones x denom
  broadcast matmul -> PSUM, fast reciprocal -> SBUF, multiply into attnT.
  bq/bk applied as per-partition adds during the PSUM->SBUF copy; bv as a
  K=1 rank-1 matmul update; bo added on host.
"""

import os as _os
import sys

if "/opt/trn_rl_repo" not in sys.path:
    sys.path.insert(0, "/opt/trn_rl_repo")

import numpy as np

import concourse.bass as bass
import concourse.tile as tile
from concourse import bacc, mybir
from concourse.bass_utils import run_bass_kernel_spmd

F32 = mybir.dt.float32
F16 = mybir.dt.float16
BF16 = mybir.dt.bfloat16
EXP = mybir.ActivationFunctionType.Exp

# Matmul operand dtype (must be 2-byte: 1 PE cycle/row and fits SBUF budget)
MM_DT = {"f16": F16, "bf16": BF16}[_os.environ.get("MM_DT", "f16")]

B, T, C = 4, 2048, 1024
H, D = 16, 64
HPC = 8          # heads per core
HD = HPC * D     # 512: per-core head-dim slab
N_CORES = 8
SCALE = D ** -0.5

KO = C // 128        # 8 contraction tiles over C
TC = T // 512        # 4 t-chunks of 512
NQ = T // 512        # 4 query blocks per head
NKT = T // 128       # 16 key tiles
HDO = HD // 128      # 4 hd tiles


def _np_of(dt):
    return np.dtype(mybir.dt.np(dt))


def build_nc(mm_dt=None):
    mm_dt = mm_dt or MM_DT
    nc = bacc.Bacc("TRN2", target_bir_lowering=False, debug=False)

    xT = nc.dram_tensor("xT", [C, T], mm_dt, kind="ExternalInput")
    wq = nc.dram_tensor("wq", [C, HD], mm_dt, kind="ExternalInput")
    wk = nc.dram_tensor("wk", [C, HD], mm_dt, kind="ExternalInput")
    wv = nc.dram_tensor("wv", [C, HD], mm_dt, kind="ExternalInput")
    wo = nc.dram_tensor("wo", [HD, C], mm_dt, kind="ExternalInput")
    # bq/bk as [128, HD//128] columns (per-partition adds in qkvT layout)
    bqc = nc.dram_tensor("bqc", [128, HD // 128], F32, kind="ExternalInput")
    bkc = nc.dram_tensor("bkc", [128, HD // 128], F32, kind="ExternalInput")
    bv = nc.dram_tensor("bv", [1, HD], mm_dt, kind="ExternalInput")
    tri = nc.dram_tensor("tri", [128, 128], mm_dt, kind="ExternalInput")
    out = nc.dram_tensor("out", [T, C], F32, kind="ExternalOutput")

    with tile.TileContext(nc) as tc:
        const = tc.alloc_tile_pool(name="const", bufs=1)
        persist = tc.alloc_tile_pool(name="persist", bufs=1)
        # PSUM banks: mm [128,1024]=2 banks x2 bufs + aug x2 + bc x1 = 7 of 8
        psum = tc.alloc_tile_pool(name="psum", bufs=2, space="PSUM")
        psum_aug = tc.alloc_tile_pool(name="psum_aug", bufs=2, space="PSUM")
        psum_bc = tc.alloc_tile_pool(name="psum_bc", bufs=1, space="PSUM")
        xt_pool = tc.alloc_tile_pool(name="xt", bufs=10)
        e_pool = tc.alloc_tile_pool(name="e", bufs=6)
        r_pool = tc.alloc_tile_pool(name="r", bufs=3)
        o_pool = tc.alloc_tile_pool(name="o", bufs=3)

        # --- constants ---
        ones_f = const.tile([1, 512], F32)
        ones_r = const.tile([1, 512], mm_dt)
        nc.vector.memset(ones_f[:], 1.0)
        nc.vector.tensor_copy(ones_r[:], ones_f[:])
        ones_col_f = const.tile([128, 1], F32)
        nc.vector.memset(ones_col_f[:], 1.0)
        tri_sb = const.tile([128, 128], mm_dt)
        nc.sync.dma_start(tri_sb[:], tri[:, :])
        bqc_sb = const.tile([128, HD // 128], F32)
        bkc_sb = const.tile([128, HD // 128], F32)
        bv_sb = const.tile([1, HD], mm_dt)
        nc.sync.dma_start(bqc_sb[:], bqc[:, :])
        nc.sync.dma_start(bkc_sb[:], bkc[:, :])
        nc.sync.dma_start(bv_sb[:], bv[:, :])

        # --- persistent tensors (split per t-chunk so attention blocks only
        # depend on the chunks they read) ---
        qT_sb = [persist.tile([128, HDO, 512], mm_dt, name=f"qT{_t}") for _t in range(TC)]
        kT_sb = [persist.tile([128, HDO, 512], mm_dt, name=f"kT{_t}") for _t in range(TC)]
        # [tpart, ktile-in-chunk, head, d|1]
        v_sb = [persist.tile([128, 4, HPC, D + 1], mm_dt, name=f"v{_t}") for _t in range(TC)]
        for vt in v_sb:
            nc.vector.tensor_copy(
                vt[:, :, :, D], ones_col_f[:, 0:1].to_broadcast([128, 4, HPC])
            )
        attnT_sb = persist.tile([128, HDO, T], mm_dt)
        wq_sb = persist.tile([128, KO, HD], mm_dt)
        wk_sb = persist.tile([128, KO, HD], mm_dt)
        wv_sb = persist.tile([128, KO, HD], mm_dt)
        wo_sb = persist.tile([128, HDO, C], mm_dt)
        for w_sb, w_d in ((wq_sb, wq), (wk_sb, wk), (wv_sb, wv), (wo_sb, wo)):
            kos = w_sb.shape[1]
            for ko in range(kos):
                nc.sync.dma_start(w_sb[:, ko], w_d[ko * 128 : (ko + 1) * 128, :])

        # --- phase-1 chunk emission: qT/kT/v for t in [tc4*512, tc4*512+512)
        # Emitted as a list of closures so chunks can interleave with
        # attention blocks in PE program order.
        def ph1_units(tc4):
            ts_ = slice(tc4 * 512, (tc4 + 1) * 512)
            xt = []

            def load_xt():
                for ko in range(KO):
                    t_ = xt_pool.tile([128, 512], mm_dt, tag="xt")
                    nc.sync.dma_start(t_[:], xT[ko * 128 : (ko + 1) * 128, ts_])
                    xt.append(t_)

            units = [load_xt]

            def qk_unit(w_sb, b_sb, dst, i):
                def emit():
                    cs = slice(i * 128, (i + 1) * 128)
                    ps = psum.tile([128, 1024], F32, tag="mm")
                    for ko in range(KO):
                        nc.tensor.matmul(
                            ps[:, 0:512], w_sb[:, ko, cs], xt[ko][:],
                            start=(ko == 0), stop=(ko == KO - 1),
                        )
                    nc.vector.tensor_scalar_add(
                        dst[:, i, :], ps[:, 0:512], b_sb[:, i : i + 1]
                    )
                return emit

            def v_unit(s):
                def emit():
                    ps = psum.tile([128, 1024], F32, tag="mm")
                    for ko in range(KO):
                        nc.tensor.matmul(
                            ps[:, 0:512],
                            xt[ko][:, s * 128 : (s + 1) * 128], wv_sb[:, ko, :],
                            start=(ko == 0), stop=False,
                        )
                    nc.tensor.matmul(
                        ps[:, 0:512], ones_r[0:1, 0:128], bv_sb[0:1, :],
                        start=False, stop=True, skip_group_check=True,
                    )
                    nc.scalar.copy(
                        v_sb[tc4][:, s, :, 0:D],
                        ps[:, 0:512].rearrange("p (h d) -> p h d", h=HPC),
                    )
                return emit

            for i in range(HDO):
                units.append(qk_unit(wq_sb, bqc_sb, qT_sb[tc4], i))
                units.append(qk_unit(wk_sb, bkc_sb, kT_sb[tc4], i))
            for s in range(4):
                units.append(v_unit(s))
            return units

        # --- attention block (h, q): uses qT chunk q, kT/v chunks <= q ---
        pending = None  # (aug, drow, pr, co, q) awaiting normalization

        def flush_norm():
            nonlocal pending
            if pending is None:
                return
            aug, drow, pr, co, q = pending
            bc = psum_bc.tile([64, 512], F32, tag="bc")
            nc.tensor.matmul(bc[:], ones_r[0:1, 0:64], drow[:], start=True, stop=True)
            rec = r_pool.tile([64, 512], F32, tag="rec")
            # ~4e-6 relerr, ~5x faster than exact reciprocal; denom >= ~0.04
            nc.vector.reciprocal_approx_fast(rec[:], bc[:])
            nc.vector.tensor_mul(
                attnT_sb[pr : pr + 64, co, q * 512 : (q + 1) * 512],
                aug[0:D, :], rec[:],
            )
            pending = None

        def attn_block(h, q):
            nonlocal pending
            co, pr = h // 2, (h % 2) * 64
            qTh = qT_sb[q][pr : pr + 64, co, :]
            jmax = 4 * q + 3
            aug = psum_aug.tile([D + 1, 512], F32, tag="aug")
            j = 0
            while j <= jmax:
                kTh = kT_sb[j // 4][pr : pr + 64, co, :]
                if j + 1 < 4 * q and j % 4 < 3:
                    # two full key tiles in one chunk: one 1024-wide exp
                    ps = psum.tile([128, 1024], F32, tag="mm")
                    e = e_pool.tile([128, 1024], mm_dt, tag="e")
                    for u in range(2):
                        nc.tensor.matmul(
                            ps[:, u * 512 : (u + 1) * 512],
                            kTh[:, (j + u) % 4 * 128 : ((j + u) % 4 + 1) * 128],
                            qTh[:],
                            start=True, stop=True, skip_group_check=True,
                        )
                    nc.scalar.activation(e[:], ps[:], EXP, scale=SCALE)
                    for u in range(2):
                        nc.tensor.matmul(
                            aug[:], v_sb[(j + u) // 4][:, (j + u) % 4, h, :],
                            e[:, u * 512 : (u + 1) * 512],
                            start=(j + u == 0), stop=False,
                            skip_group_check=True,
                        )
                    j += 2
                    continue
                diag = j >= 4 * q
                c0 = 128 * (j - 4 * q) if diag else 0
                ncol = 512 - c0
                ps = psum.tile([128, 1024], F32, tag="mm")
                nc.tensor.matmul(
                    ps[:, :ncol],
                    kTh[:, j % 4 * 128 : (j % 4 + 1) * 128],
                    qTh[:, c0:512],
                    start=True, stop=True,
                )
                e = e_pool.tile([128, 1024], mm_dt, tag="e")
                nc.scalar.activation(e[:, :ncol], ps[:, :ncol], EXP, scale=SCALE)
                if diag:
                    nc.vector.tensor_mul(e[:, 0:128], e[:, 0:128], tri_sb[:])
                nc.tensor.matmul(
                    aug[:, c0:], v_sb[j // 4][:, j % 4, h, :], e[:, :ncol],
                    start=(j == 0), stop=(j == jmax), skip_group_check=True,
                )
                j += 1
            drow = r_pool.tile([1, 512], mm_dt, tag="drow")
            with nc.allow_low_precision(reason="softmax denom rounding"):
                nc.vector.tensor_copy(drow[:], aug[D : D + 1, :])
            flush_norm()
            pending = (aug, drow, pr, co, q)

        # --- emission: chunk 0, then per-q rounds with next chunk interleaved
        for u in ph1_units(0):
            u()
        for q in range(NQ):
            filler = ph1_units(q + 1) if q + 1 < TC else []
            fi = 0
            for h in range(HPC):
                attn_block(h, q)
                # spread next chunk's units across the 8 heads
                take = (len(filler) - fi) // (HPC - h) if h < HPC else 0
                for _ in range(take):
                    filler[fi]()
                    fi += 1
            while fi < len(filler):
                filler[fi]()
                fi += 1
        flush_norm()

        # --- tail: output projection (PSUM-accumulated over hd tiles) ---
        for tt in range(NKT):
            ps = psum.tile([128, 1024], F32, tag="mm")
            for cc in range(2):
                for ko in range(HDO):
                    nc.tensor.matmul(
                        ps[:, cc * 512 : (cc + 1) * 512],
                        attnT_sb[:, ko, tt * 128 : (tt + 1) * 128],
                        wo_sb[:, ko, cc * 512 : (cc + 1) * 512],
                        start=(ko == 0), stop=(ko == HDO - 1),
                        skip_group_check=True,
                    )
            osb = o_pool.tile([128, 1024], F32, tag="osb")
            nc.vector.tensor_copy(osb[:], ps[:])
            nc.sync.dma_start(out[tt * 128 : (tt + 1) * 128, :], osb[:])

        o_pool.release()
        r_pool.release()
        e_pool.release()
        xt_pool.release()
        psum_bc.release()
        psum_aug.release()
        psum.release()
        persist.release()
        const.release()

    nc.finalize()
    return nc


_NC_CACHE = {}


def _get_nc(mm_dt=None):
    key = str(mm_dt or MM_DT)
    if key not in _NC_CACHE:
        _NC_CACHE[key] = build_nc(mm_dt)
    return _NC_CACHE[key]


def make_in_maps(x, Wqkv, bqkv, Wo, mm_dt=None):
    mdt = _np_of(mm_dt or MM_DT)
    x = np.asarray(x, dtype=np.float32)
    Wqkv = np.asarray(Wqkv, dtype=np.float32)
    bqkv = np.asarray(bqkv, dtype=np.float32)
    Wo = np.asarray(Wo, dtype=np.float32)

    w3 = Wqkv.reshape(C, 3, H, D)
    b3 = bqkv.reshape(3, H, D)
    wo4 = Wo.reshape(H, D, C)
    tri = np.triu(np.ones((128, 128), dtype=np.float32))

    in_maps = []
    for c in range(N_CORES):
        b, g = c // 2, c % 2
        hs = slice(g * HPC, (g + 1) * HPC)
        bq = b3[0, hs].reshape(HD)
        bk = b3[1, hs].reshape(HD)
        in_maps.append({
            "xT": np.ascontiguousarray(x[b].T).astype(mdt),
            "wq": np.ascontiguousarray(w3[:, 0, hs, :].reshape(C, HD)).astype(mdt),
            "wk": np.ascontiguousarray(w3[:, 1, hs, :].reshape(C, HD)).astype(mdt),
            "wv": np.ascontiguousarray(w3[:, 2, hs, :].reshape(C, HD)).astype(mdt),
            "wo": np.ascontiguousarray(wo4[hs].reshape(HD, C)).astype(mdt),
            "bqc": np.ascontiguousarray(bq.reshape(HD // 128, 128).T).astype(np.float32),
            "bkc": np.ascontiguousarray(bk.reshape(HD // 128, 128).T).astype(np.float32),
            "bv": b3[2, hs].reshape(1, HD).astype(mdt),
            "tri": tri.astype(mdt),
        })
    return in_maps


def run(x, Wqkv, bqkv, Wo, bo, mm_dt=None, **spmd_kwargs):
    nc = _get_nc(mm_dt)
    in_maps = make_in_maps(x, Wqkv, bqkv, Wo, mm_dt=mm_dt)
    res = run_bass_kernel_spmd(nc, in_maps, core_ids=list(range(N_CORES)),
                               **spmd_kwargs)
    bo = np.asarray(bo, dtype=np.float32)
    out = np.empty((B, T, C), dtype=np.float32)
    for b in range(B):
        out[b] = res.results[2 * b]["out"] + res.results[2 * b + 1]["out"] + bo
    return out, res


def kernel(x, Wqkv, bqkv, Wo, bo):
    out, _ = run(x, Wqkv, bqkv, Wo, bo)
    return out


# revision 16
# speedup vs baseline: 1.4603x; 1.1262x over previous
"""Causal self-attention Trainium2 Bass kernel.

Problem: B=4, T=2048, C=1024, H=16 heads, head_dim=64, fp32.
    qkv = x @ Wqkv + bqkv ; per-head causal softmax attention ; out = attn @ Wo + bo

Sharding (8 NeuronCores): core c -> (batch b = c//2, head-group g = c%2).
Each core computes qkv for its batch restricted to its 8 heads, attention for
those heads, and a partial output projection against its 512 rows of Wo.
The host sums the two partials of each batch pair (the tensor-parallel
all-reduce), adds bo, and stacks batches.

On-core dataflow (matmul dtype MM_DT = fp16 by default; PSUM accumulation is
always fp32):

  The kernel is emitted QUERY-BLOCK-MAJOR so that projection (phase-1) matmul
  work interleaves with attention (phase-2) work on the PE.  Attention
  couples PE->ACT->PE (scores -> exp -> weighted sum), and the exp stream on
  the Scalar engine is slightly slower than the PE's attention work, so a
  pure attention phase starves the PE in sub-microsecond gaps; the PE clock
  monitor then halves the PE clock (K=4/8 gating needs ~3.4us of
  uninterrupted work to re-warm).  Interleaving the independent qkv
  projection matmuls keeps the PE saturated.

    round tc=0:  qT/kT/v chunk 0 (t in [0,512))
    round q:     attention blocks (h, q) for all 8 heads,
                 interleaved with qT/kT/v chunk q+1
    tail:        out_partial[t,c] = attnT-tile^T @ Wo-rows (PSUM-accumulated)

  Attention per (head, 512-query block): S_T[k,q] = kT-tile^T @ qT, exp via
  ACT (scale=1/8 folded in; scores bounded ~|3.2| so no max subtraction;
  full key-tile pairs share one 1024-wide exp), causal masking via a host
  triangular tile + sub-range accumulation, attnT_aug = [v|1]^T @ expS_T
  accumulated over key tiles (row 64 = softmax denominator).  Normalization
  is software-pipelined one block behind: denom row -> SBUF, ones x denom
  broadcast matmul -> PSUM, fast reciprocal -> SBUF, multiply into attnT.
  bq/bk applied as per-partition adds during the PSUM->SBUF copy; bv as a
  K=1 rank-1 matmul update; bo added on host.
"""

import os as _os
import sys

if "/opt/trn_rl_repo" not in sys.path:
    sys.path.insert(0, "/opt/trn_rl_repo")

import numpy as np

import concourse.bass as bass
import concourse.tile as tile
from concourse import bacc, mybir
from concourse.bass_utils import run_bass_kernel_spmd

F32 = mybir.dt.float32
F16 = mybir.dt.float16
BF16 = mybir.dt.bfloat16
EXP = mybir.ActivationFunctionType.Exp

# Matmul operand dtype (must be 2-byte: 1 PE cycle/row and fits SBUF budget)
MM_DT = {"f16": F16, "bf16": BF16}[_os.environ.get("MM_DT", "f16")]

B, T, C = 4, 2048, 1024
H, D = 16, 64
HPC = 8          # heads per core
HD = HPC * D     # 512: per-core head-dim slab
N_CORES = 8
SCALE = D ** -0.5

KO = C // 128        # 8 contraction tiles over C
TC = T // 512        # 4 t-chunks of 512
NQ = T // 512        # 4 query blocks per head
NKT = T // 128       # 16 key tiles
HDO = HD // 128      # 4 hd tiles


def _np_of(dt):
    return np.dtype(mybir.dt.np(dt))


def build_nc(mm_dt=None):
    mm_dt = mm_dt or MM_DT
    nc = bacc.Bacc("TRN2", target_bir_lowering=False, debug=False)

    xT = nc.dram_tensor("xT", [C, T], mm_dt, kind="ExternalInput")
    wq = nc.dram_tensor("wq", [C, HD], mm_dt, kind="ExternalInput")
    wk = nc.dram_tensor("wk", [C, HD], mm_dt, kind="ExternalInput")
    wv = nc.dram_tensor("wv", [C, HD], mm_dt, kind="ExternalInput")
    wo = nc.dram_tensor("wo", [HD, C], mm_dt, kind="ExternalInput")
    # bq/bk as [128, HD//128] columns (per-partition adds in qkvT layout)
    bqc = nc.dram_tensor("bqc", [128, HD // 128], F32, kind="ExternalInput")
    bkc = nc.dram_tensor("bkc", [128, HD // 128], F32, kind="ExternalInput")
    bv = nc.dram_tensor("bv", [1, HD], mm_dt, kind="ExternalInput")
    tri = nc.dram_tensor("tri", [128, 128], mm_dt, kind="ExternalInput")
    out = nc.dram_tensor("out", [T, C], F32, kind="ExternalOutput")

    with tile.TileContext(nc) as tc:
        const = tc.alloc_tile_pool(name="const", bufs=1)
        persist = tc.alloc_tile_pool(name="persist", bufs=1)
        # PSUM banks: mm [128,1024]=2 banks x2 bufs + aug x2 + bc x1 = 7 of 8
        psum = tc.alloc_tile_pool(name="psum", bufs=2, space="PSUM")
        psum_aug = tc.alloc_tile_pool(name="psum_aug", bufs=2, space="PSUM")
        psum_bc = tc.alloc_tile_pool(name="psum_bc", bufs=1, space="PSUM")
        xt_pool = tc.alloc_tile_pool(name="xt", bufs=10)
        e_pool = tc.alloc_tile_pool(name="e", bufs=6)
        r_pool = tc.alloc_tile_pool(name="r", bufs=3)
        o_pool = tc.alloc_tile_pool(name="o", bufs=3)

        # --- constants ---
        ones_f = const.tile([1, 512], F32)
        ones_r = const.tile([1, 512], mm_dt)
        nc.vector.memset(ones_f[:], 1.0)
        nc.vector.tensor_copy(ones_r[:], ones_f[:])
        ones_col_f = const.tile([128, 1], F32)
        nc.vector.memset(ones_col_f[:], 1.0)
        tri_sb = const.tile([128, 128], mm_dt)
        nc.sync.dma_start(tri_sb[:], tri[:, :])
        bqc_sb = const.tile([128, HD // 128], F32)
        bkc_sb = const.tile([128, HD // 128], F32)
        bv_sb = const.tile([1, HD], mm_dt)
        nc.sync.dma_start(bqc_sb[:], bqc[:, :])
        nc.sync.dma_start(bkc_sb[:], bkc[:, :])
        nc.sync.dma_start(bv_sb[:], bv[:, :])

        # --- persistent tensors (split per t-chunk so attention blocks only
        # depend on the chunks they read) ---
        qT_sb = [persist.tile([128, HDO, 512], mm_dt, name=f"qT{_t}") for _t in range(TC)]
        kT_sb = [persist.tile([128, HDO, 512], mm_dt, name=f"kT{_t}") for _t in range(TC)]
        # [tpart, ktile-in-chunk, head, d|1]
        v_sb = [persist.tile([128, 4, HPC, D + 1], mm_dt, name=f"v{_t}") for _t in range(TC)]
        for vt in v_sb:
            nc.vector.tensor_copy(
                vt[:, :, :, D], ones_col_f[:, 0:1].to_broadcast([128, 4, HPC])
            )
        attnT_sb = persist.tile([128, HDO, T], mm_dt)
        wq_sb = persist.tile([128, KO, HD], mm_dt)
        wk_sb = persist.tile([128, KO, HD], mm_dt)
        wv_sb = persist.tile([128, KO, HD], mm_dt)
        wo_sb = persist.tile([128, HDO, C], mm_dt)
        # weights are loaded via GpSimd-issued DMAs so they don't serialize
        # behind the x-chunk loads on the sync engine's issue pipe
        for w_sb, w_d in ((wq_sb, wq), (wk_sb, wk), (wv_sb, wv), (wo_sb, wo)):
            kos = w_sb.shape[1]
            for ko in range(kos):
                nc.gpsimd.dma_start(w_sb[:, ko], w_d[ko * 128 : (ko + 1) * 128, :])

        # --- phase-1 chunk emission: qT/kT/v for t in [tc4*512, tc4*512+512)
        # Emitted as a list of closures so chunks can interleave with
        # attention blocks in PE program order.
        def ph1_units(tc4):
            ts_ = slice(tc4 * 512, (tc4 + 1) * 512)
            xt = []

            def load_xt():
                for ko in range(KO):
                    t_ = xt_pool.tile([128, 512], mm_dt, tag="xt")
                    nc.sync.dma_start(t_[:], xT[ko * 128 : (ko + 1) * 128, ts_])
                    xt.append(t_)

            units = [load_xt]

            def qk_unit(w_sb, b_sb, dst, i):
                def emit():
                    cs = slice(i * 128, (i + 1) * 128)
                    ps = psum.tile([128, 1024], F32, tag="mm")
                    for ko in range(KO):
                        nc.tensor.matmul(
                            ps[:, 0:512], w_sb[:, ko, cs], xt[ko][:],
                            start=(ko == 0), stop=(ko == KO - 1),
                        )
                    nc.vector.tensor_scalar_add(
                        dst[:, i, :], ps[:, 0:512], b_sb[:, i : i + 1]
                    )
                return emit

            def v_unit(s):
                def emit():
                    ps = psum.tile([128, 1024], F32, tag="mm")
                    for ko in range(KO):
                        nc.tensor.matmul(
                            ps[:, 0:512],
                            xt[ko][:, s * 128 : (s + 1) * 128], wv_sb[:, ko, :],
                            start=(ko == 0), stop=False,
                        )
                    nc.tensor.matmul(
                        ps[:, 0:512], ones_r[0:1, 0:128], bv_sb[0:1, :],
                        start=False, stop=True, skip_group_check=True,
                    )
                    nc.scalar.copy(
                        v_sb[tc4][:, s, :, 0:D],
                        ps[:, 0:512].rearrange("p (h d) -> p h d", h=HPC),
                    )
                return emit

            for i in range(HDO):
                units.append(qk_unit(wq_sb, bqc_sb, qT_sb[tc4], i))
                units.append(qk_unit(wk_sb, bkc_sb, kT_sb[tc4], i))
            for s in range(4):
                units.append(v_unit(s))
            return units

        # --- attention block (h, q): uses qT chunk q, kT/v chunks <= q ---
        pending = None  # (aug, drow, pr, co, q) awaiting normalization

        def flush_norm():
            nonlocal pending
            if pending is None:
                return
            aug, drow, pr, co, q = pending
            bc = psum_bc.tile([64, 512], F32, tag="bc")
            nc.tensor.matmul(bc[:], ones_r[0:1, 0:64], drow[:], start=True, stop=True)
            rec = r_pool.tile([64, 512], F32, tag="rec")
            # ~4e-6 relerr, ~5x faster than exact reciprocal; denom >= ~0.04
            nc.vector.reciprocal_approx_fast(rec[:], bc[:])
            nc.vector.tensor_mul(
                attnT_sb[pr : pr + 64, co, q * 512 : (q + 1) * 512],
                aug[0:D, :], rec[:],
            )
            pending = None

        def attn_block(h, q):
            nonlocal pending
            co, pr = h // 2, (h % 2) * 64
            qTh = qT_sb[q][pr : pr + 64, co, :]
            jmax = 4 * q + 3
            aug = psum_aug.tile([D + 1, 512], F32, tag="aug")
            j = 0
            while j <= jmax:
                kTh = kT_sb[j // 4][pr : pr + 64, co, :]
                if j + 1 < 4 * q and j % 4 < 3:
                    # two full key tiles in one chunk: one 1024-wide exp
                    ps = psum.tile([128, 1024], F32, tag="mm")
                    e = e_pool.tile([128, 1024], mm_dt, tag="e")
                    for u in range(2):
                        nc.tensor.matmul(
                            ps[:, u * 512 : (u + 1) * 512],
                            kTh[:, (j + u) % 4 * 128 : ((j + u) % 4 + 1) * 128],
                            qTh[:],
                            start=True, stop=True, skip_group_check=True,
                        )
                    nc.scalar.activation(e[:], ps[:], EXP, scale=SCALE)
                    for u in range(2):
                        nc.tensor.matmul(
                            aug[:], v_sb[(j + u) // 4][:, (j + u) % 4, h, :],
                            e[:, u * 512 : (u + 1) * 512],
                            start=(j + u == 0), stop=False,
                            skip_group_check=True,
                        )
                    j += 2
                    continue
                diag = j >= 4 * q
                c0 = 128 * (j - 4 * q) if diag else 0
                ncol = 512 - c0
                ps = psum.tile([128, 1024], F32, tag="mm")
                nc.tensor.matmul(
                    ps[:, :ncol],
                    kTh[:, j % 4 * 128 : (j % 4 + 1) * 128],
                    qTh[:, c0:512],
                    start=True, stop=True,
                )
                e = e_pool.tile([128, 1024], mm_dt, tag="e")
                nc.scalar.activation(e[:, :ncol], ps[:, :ncol], EXP, scale=SCALE)
                if diag:
                    nc.vector.tensor_mul(e[:, 0:128], e[:, 0:128], tri_sb[:])
                nc.tensor.matmul(
                    aug[:, c0:], v_sb[j // 4][:, j % 4, h, :], e[:, :ncol],
                    start=(j == 0), stop=(j == jmax), skip_group_check=True,
                )
                j += 1
            drow = r_pool.tile([1, 512], mm_dt, tag="drow")
            with nc.allow_low_precision(reason="softmax denom rounding"):
                nc.vector.tensor_copy(drow[:], aug[D : D + 1, :])
            flush_norm()
            pending = (aug, drow, pr, co, q)

        # --- tail unit: output projection for one t-tile (PSUM-accumulated
        # over hd tiles); ready once round tt//4 is normalized ---
        def tail_unit(tt):
            def emit():
                ps = psum.tile([128, 1024], F32, tag="mm")
                for cc in range(2):
                    for ko in range(HDO):
                        nc.tensor.matmul(
                            ps[:, cc * 512 : (cc + 1) * 512],
                            attnT_sb[:, ko, tt * 128 : (tt + 1) * 128],
                            wo_sb[:, ko, cc * 512 : (cc + 1) * 512],
                            start=(ko == 0), stop=(ko == HDO - 1),
                            skip_group_check=True,
                        )
                osb = o_pool.tile([128, 1024], F32, tag="osb")
                nc.vector.tensor_copy(osb[:], ps[:])
                nc.sync.dma_start(out[tt * 128 : (tt + 1) * 128, :], osb[:])
            return emit

        # --- emission: chunk 0, then per-q rounds; rounds 0-2 interleave the
        # next projection chunk, round 3 interleaves ready tail tiles ---
        units0 = ph1_units(0)
        units0[0]()  # x chunk-0 loads go out first
        for u in units0[1:]:
            u()
        for q in range(NQ):
            if q + 1 < TC:
                filler = ph1_units(q + 1)
            else:
                filler = [tail_unit(tt) for tt in range(12)]
            fi = 0
            for h in range(HPC):
                attn_block(h, q)
                # spread filler units across the 8 heads
                take = (len(filler) - fi) // (HPC - h) if h < HPC else 0
                for _ in range(take):
                    filler[fi]()
                    fi += 1
            while fi < len(filler):
                filler[fi]()
                fi += 1
        flush_norm()
        for tt in range(12, NKT):
            tail_unit(tt)()

        o_pool.release()
        r_pool.release()
        e_pool.release()
        xt_pool.release()
        psum_bc.release()
        psum_aug.release()
        psum.release()
        persist.release()
        const.release()

    nc.finalize()
    return nc


_NC_CACHE = {}


def _get_nc(mm_dt=None):
    key = str(mm_dt or MM_DT)
    if key not in _NC_CACHE:
        _NC_CACHE[key] = build_nc(mm_dt)
    return _NC_CACHE[key]


def make_in_maps(x, Wqkv, bqkv, Wo, mm_dt=None):
    mdt = _np_of(mm_dt or MM_DT)
    x = np.asarray(x, dtype=np.float32)
    Wqkv = np.asarray(Wqkv, dtype=np.float32)
    bqkv = np.asarray(bqkv, dtype=np.float32)
    Wo = np.asarray(Wo, dtype=np.float32)

    w3 = Wqkv.reshape(C, 3, H, D)
    b3 = bqkv.reshape(3, H, D)
    wo4 = Wo.reshape(H, D, C)
    tri = np.triu(np.ones((128, 128), dtype=np.float32))

    in_maps = []
    for c in range(N_CORES):
        b, g = c // 2, c % 2
        hs = slice(g * HPC, (g + 1) * HPC)
        bq = b3[0, hs].reshape(HD)
        bk = b3[1, hs].reshape(HD)
        in_maps.append({
            "xT": np.ascontiguousarray(x[b].T).astype(mdt),
            "wq": np.ascontiguousarray(w3[:, 0, hs, :].reshape(C, HD)).astype(mdt),
            "wk": np.ascontiguousarray(w3[:, 1, hs, :].reshape(C, HD)).astype(mdt),
            "wv": np.ascontiguousarray(w3[:, 2, hs, :].reshape(C, HD)).astype(mdt),
            "wo": np.ascontiguousarray(wo4[hs].reshape(HD, C)).astype(mdt),
            "bqc": np.ascontiguousarray(bq.reshape(HD // 128, 128).T).astype(np.float32),
            "bkc": np.ascontiguousarray(bk.reshape(HD // 128, 128).T).astype(np.float32),
            "bv": b3[2, hs].reshape(1, HD).astype(mdt),
            "tri": tri.astype(mdt),
        })
    return in_maps


def run(x, Wqkv, bqkv, Wo, bo, mm_dt=None, **spmd_kwargs):
    nc = _get_nc(mm_dt)
    in_maps = make_in_maps(x, Wqkv, bqkv, Wo, mm_dt=mm_dt)
    res = run_bass_kernel_spmd(nc, in_maps, core_ids=list(range(N_CORES)),
                               **spmd_kwargs)
    bo = np.asarray(bo, dtype=np.float32)
    out = np.empty((B, T, C), dtype=np.float32)
    for b in range(B):
        out[b] = res.results[2 * b]["out"] + res.results[2 * b + 1]["out"] + bo
    return out, res


def kernel(x, Wqkv, bqkv, Wo, bo):
    out, _ = run(x, Wqkv, bqkv, Wo, bo)
    return out


# revision 20
# speedup vs baseline: 1.4975x; 1.0255x over previous
"""Causal self-attention Trainium2 Bass kernel.

Problem: B=4, T=2048, C=1024, H=16 heads, head_dim=64, fp32.
    qkv = x @ Wqkv + bqkv ; per-head causal softmax attention ; out = attn @ Wo + bo

Sharding (8 NeuronCores): core c -> (batch b = c//2, head-group g = c%2).
Each core computes qkv for its batch restricted to its 8 heads, attention for
those heads, and a partial output projection against its 512 rows of Wo.
The host sums the two partials of each batch pair (the tensor-parallel
all-reduce), adds bo, and stacks batches.

On-core dataflow (matmul dtype MM_DT = fp16 by default; PSUM accumulation is
always fp32):

  The kernel is emitted QUERY-BLOCK-MAJOR so that projection (phase-1) matmul
  work interleaves with attention (phase-2) work on the PE.  Attention
  couples PE->ACT->PE (scores -> exp -> weighted sum), and the exp stream on
  the Scalar engine is slightly slower than the PE's attention work, so a
  pure attention phase starves the PE in sub-microsecond gaps; the PE clock
  monitor then halves the PE clock (K=4/8 gating needs ~3.4us of
  uninterrupted work to re-warm).  Interleaving the independent qkv
  projection matmuls keeps the PE saturated.

    round tc=0:  qT/kT/v chunk 0 (t in [0,512))
    round q:     attention blocks (h, q) for all 8 heads,
                 interleaved with qT/kT/v chunk q+1
    tail:        out_partial[t,c] = attnT-tile^T @ Wo-rows (PSUM-accumulated)

  Attention per (head, 512-query block): S_T[k,q] = kT-tile^T @ qT, exp via
  ACT (scale=1/8 folded in; scores bounded ~|3.2| so no max subtraction;
  full key-tile pairs share one 1024-wide exp), causal masking via a host
  triangular tile + sub-range accumulation, attnT_aug = [v|1]^T @ expS_T
  accumulated over key tiles (row 64 = softmax denominator).  Normalization
  is software-pipelined one block behind: denom row -> SBUF, ones x denom
  broadcast matmul -> PSUM, fast reciprocal -> SBUF, multiply into attnT.
  bq/bk applied as per-partition adds during the PSUM->SBUF copy; bv as a
  K=1 rank-1 matmul update; bo added on host.
"""

import os as _os
import sys

if "/opt/trn_rl_repo" not in sys.path:
    sys.path.insert(0, "/opt/trn_rl_repo")

import numpy as np

import concourse.bass as bass
import concourse.tile as tile
from concourse import bacc, mybir
from concourse.bass_utils import run_bass_kernel_spmd

F32 = mybir.dt.float32
F16 = mybir.dt.float16
BF16 = mybir.dt.bfloat16
EXP = mybir.ActivationFunctionType.Exp

# Matmul operand dtype (must be 2-byte: 1 PE cycle/row and fits SBUF budget)
MM_DT = {"f16": F16, "bf16": BF16}[_os.environ.get("MM_DT", "f16")]

B, T, C = 4, 2048, 1024
H, D = 16, 64
HPC = 8          # heads per core
HD = HPC * D     # 512: per-core head-dim slab
N_CORES = 8
SCALE = D ** -0.5

KO = C // 128        # 8 contraction tiles over C
TC = T // 512        # 4 t-chunks of 512
NQ = T // 512        # 4 query blocks per head
NKT = T // 128       # 16 key tiles
HDO = HD // 128      # 4 hd tiles


def _np_of(dt):
    return np.dtype(mybir.dt.np(dt))


def build_nc(mm_dt=None):
    mm_dt = mm_dt or MM_DT
    nc = bacc.Bacc("TRN2", target_bir_lowering=False, debug=False)

    xT = nc.dram_tensor("xT", [C, T], mm_dt, kind="ExternalInput")
    wq = nc.dram_tensor("wq", [C, HD], mm_dt, kind="ExternalInput")
    wk = nc.dram_tensor("wk", [C, HD], mm_dt, kind="ExternalInput")
    wv = nc.dram_tensor("wv", [C, HD], mm_dt, kind="ExternalInput")
    wo = nc.dram_tensor("wo", [HD, C], mm_dt, kind="ExternalInput")
    # bq/bk as [128, HD//128] columns (per-partition adds in qkvT layout)
    bqc = nc.dram_tensor("bqc", [128, HD // 128], F32, kind="ExternalInput")
    bkc = nc.dram_tensor("bkc", [128, HD // 128], F32, kind="ExternalInput")
    bv = nc.dram_tensor("bv", [1, HD], mm_dt, kind="ExternalInput")
    tri = nc.dram_tensor("tri", [128, 128], mm_dt, kind="ExternalInput")
    out = nc.dram_tensor("out", [T, C], F32, kind="ExternalOutput")

    with tile.TileContext(nc) as tc:
        const = tc.alloc_tile_pool(name="const", bufs=1)
        persist = tc.alloc_tile_pool(name="persist", bufs=1)
        # PSUM banks: mm [128,1024]=2 banks x2 bufs + aug x2 + bc x1 = 7 of 8
        psum = tc.alloc_tile_pool(name="psum", bufs=2, space="PSUM")
        psum_aug = tc.alloc_tile_pool(name="psum_aug", bufs=2, space="PSUM")
        psum_bc = tc.alloc_tile_pool(name="psum_bc", bufs=1, space="PSUM")
        xt_pool = tc.alloc_tile_pool(name="xt", bufs=18)
        e_pool = tc.alloc_tile_pool(name="e", bufs=8)
        r_pool = tc.alloc_tile_pool(name="r", bufs=3)
        o_pool = tc.alloc_tile_pool(name="o", bufs=3)

        # --- persistent weights, loaded first via GpSimd-issued DMAs so they
        # don't serialize behind the x-chunk loads on the sync issue pipe ---
        wq_sb = persist.tile([128, KO, HD], mm_dt)
        wk_sb = persist.tile([128, KO, HD], mm_dt)
        wv_sb = persist.tile([128, KO, HD], mm_dt)
        wo_sb = persist.tile([128, HDO, C], mm_dt)
        for w_sb, w_d in ((wq_sb, wq), (wk_sb, wk), (wv_sb, wv), (wo_sb, wo)):
            kos = w_sb.shape[1]
            for ko in range(kos):
                nc.gpsimd.dma_start(w_sb[:, ko], w_d[ko * 128 : (ko + 1) * 128, :])

        # --- constants ---
        ones_f = const.tile([1, 512], F32)
        ones_r = const.tile([1, 512], mm_dt)
        nc.vector.memset(ones_f[:], 1.0)
        nc.vector.tensor_copy(ones_r[:], ones_f[:])
        ones_col_f = const.tile([128, 1], F32)
        nc.vector.memset(ones_col_f[:], 1.0)
        tri_sb = const.tile([128, 128], mm_dt)
        nc.sync.dma_start(tri_sb[:], tri[:, :])
        bqc_sb = const.tile([128, HD // 128], F32)
        bkc_sb = const.tile([128, HD // 128], F32)
        bv_sb = const.tile([1, HD], mm_dt)
        nc.sync.dma_start(bqc_sb[:], bqc[:, :])
        nc.sync.dma_start(bkc_sb[:], bkc[:, :])
        nc.sync.dma_start(bv_sb[:], bv[:, :])

        # --- persistent tensors (split per t-chunk so attention blocks only
        # depend on the chunks they read) ---
        qT_sb = [persist.tile([128, HDO, 512], mm_dt, name=f"qT{_t}") for _t in range(TC)]
        kT_sb = [persist.tile([128, HDO, 512], mm_dt, name=f"kT{_t}") for _t in range(TC)]
        # [tpart, ktile-in-chunk, head, d|1]
        v_sb = [persist.tile([128, 4, HPC, D + 1], mm_dt, name=f"v{_t}") for _t in range(TC)]
        for vt in v_sb:
            nc.vector.tensor_copy(
                vt[:, :, :, D], ones_col_f[:, 0:1].to_broadcast([128, 4, HPC])
            )
        attnT_sb = persist.tile([128, HDO, T], mm_dt)

        # --- phase-1 chunk emission: qT/kT/v for t in [tc4*512, tc4*512+512)
        # Emitted as a list of closures so chunks can interleave with
        # attention blocks in PE program order.
        def ph1_units(tc4):
            ts_ = slice(tc4 * 512, (tc4 + 1) * 512)
            xt = []

            def load_xt():
                for ko in range(KO):
                    t_ = xt_pool.tile([128, 512], mm_dt, tag="xt")
                    nc.sync.dma_start(t_[:], xT[ko * 128 : (ko + 1) * 128, ts_])
                    xt.append(t_)

            units = [load_xt]

            def qk_unit(w_sb, b_sb, dst, i):
                def emit():
                    cs = slice(i * 128, (i + 1) * 128)
                    ps = psum.tile([128, 1024], F32, tag="mm")
                    for ko in range(KO):
                        nc.tensor.matmul(
                            ps[:, 0:512], w_sb[:, ko, cs], xt[ko][:],
                            start=(ko == 0), stop=(ko == KO - 1),
                        )
                    nc.vector.tensor_scalar_add(
                        dst[:, i, :], ps[:, 0:512], b_sb[:, i : i + 1]
                    )
                return emit

            def v_unit(s):
                def emit():
                    ps = psum.tile([128, 1024], F32, tag="mm")
                    for ko in range(KO):
                        nc.tensor.matmul(
                            ps[:, 0:512],
                            xt[ko][:, s * 128 : (s + 1) * 128], wv_sb[:, ko, :],
                            start=(ko == 0), stop=False,
                        )
                    nc.tensor.matmul(
                        ps[:, 0:512], ones_r[0:1, 0:128], bv_sb[0:1, :],
                        start=False, stop=True, skip_group_check=True,
                    )
                    nc.scalar.copy(
                        v_sb[tc4][:, s, :, 0:D],
                        ps[:, 0:512].rearrange("p (h d) -> p h d", h=HPC),
                    )
                return emit

            for i in range(HDO):
                units.append(qk_unit(wq_sb, bqc_sb, qT_sb[tc4], i))
                units.append(qk_unit(wk_sb, bkc_sb, kT_sb[tc4], i))
            for s in range(4):
                units.append(v_unit(s))
            return units

        # --- attention block (h, q): uses qT chunk q, kT/v chunks <= q ---
        pending = None  # (aug, drow, pr, co, q) awaiting normalization

        def flush_norm():
            nonlocal pending
            if pending is None:
                return
            aug, drow, pr, co, q = pending
            bc = psum_bc.tile([64, 512], F32, tag="bc")
            nc.tensor.matmul(bc[:], ones_r[0:1, 0:64], drow[:], start=True, stop=True)
            rec = r_pool.tile([64, 512], F32, tag="rec")
            # ~4e-6 relerr, ~5x faster than exact reciprocal; denom >= ~0.04
            nc.vector.reciprocal_approx_fast(rec[:], bc[:])
            nc.vector.tensor_mul(
                attnT_sb[pr : pr + 64, co, q * 512 : (q + 1) * 512],
                aug[0:D, :], rec[:],
            )
            pending = None

        def attn_block(h, q):
            nonlocal pending
            co, pr = h // 2, (h % 2) * 64
            qTh = qT_sb[q][pr : pr + 64, co, :]
            jmax = 4 * q + 3
            aug = psum_aug.tile([D + 1, 512], F32, tag="aug")
            j = 0
            while j <= jmax:
                kTh = kT_sb[j // 4][pr : pr + 64, co, :]
                if j + 1 < 4 * q and j % 4 < 3:
                    # two full key tiles in one chunk: one 1024-wide exp
                    ps = psum.tile([128, 1024], F32, tag="mm")
                    e = e_pool.tile([128, 1024], mm_dt, tag="e")
                    for u in range(2):
                        nc.tensor.matmul(
                            ps[:, u * 512 : (u + 1) * 512],
                            kTh[:, (j + u) % 4 * 128 : ((j + u) % 4 + 1) * 128],
                            qTh[:],
                            start=True, stop=True, skip_group_check=True,
                        )
                    nc.scalar.activation(e[:], ps[:], EXP, scale=SCALE)
                    for u in range(2):
                        nc.tensor.matmul(
                            aug[:], v_sb[(j + u) // 4][:, (j + u) % 4, h, :],
                            e[:, u * 512 : (u + 1) * 512],
                            start=(j + u == 0), stop=False,
                            skip_group_check=True,
                        )
                    j += 2
                    continue
                diag = j >= 4 * q
                c0 = 128 * (j - 4 * q) if diag else 0
                ncol = 512 - c0
                ps = psum.tile([128, 1024], F32, tag="mm")
                nc.tensor.matmul(
                    ps[:, :ncol],
                    kTh[:, j % 4 * 128 : (j % 4 + 1) * 128],
                    qTh[:, c0:512],
                    start=True, stop=True,
                )
                e = e_pool.tile([128, 1024], mm_dt, tag="e")
                nc.scalar.activation(e[:, :ncol], ps[:, :ncol], EXP, scale=SCALE)
                if diag:
                    nc.vector.tensor_mul(e[:, 0:128], e[:, 0:128], tri_sb[:])
                nc.tensor.matmul(
                    aug[:, c0:], v_sb[j // 4][:, j % 4, h, :], e[:, :ncol],
                    start=(j == 0), stop=(j == jmax), skip_group_check=True,
                )
                j += 1
            drow = r_pool.tile([1, 512], mm_dt, tag="drow")
            with nc.allow_low_precision(reason="softmax denom rounding"):
                nc.vector.tensor_copy(drow[:], aug[D : D + 1, :])
            flush_norm()
            pending = (aug, drow, pr, co, q)

        # --- tail unit: output projection for one t-tile (PSUM-accumulated
        # over hd tiles); ready once round tt//4 is normalized ---
        def tail_unit(tt):
            def emit():
                ps = psum.tile([128, 1024], F32, tag="mm")
                for cc in range(2):
                    for ko in range(HDO):
                        nc.tensor.matmul(
                            ps[:, cc * 512 : (cc + 1) * 512],
                            attnT_sb[:, ko, tt * 128 : (tt + 1) * 128],
                            wo_sb[:, ko, cc * 512 : (cc + 1) * 512],
                            start=(ko == 0), stop=(ko == HDO - 1),
                            skip_group_check=True,
                        )
                osb = o_pool.tile([128, 1024], F32, tag="osb")
                nc.vector.tensor_copy(osb[:], ps[:])
                nc.sync.dma_start(out[tt * 128 : (tt + 1) * 128, :], osb[:])
            return emit

        # --- emission: chunk 0, then per-q rounds; rounds 0-2 interleave the
        # next projection chunk, round 3 interleaves ready tail tiles ---
        units0 = ph1_units(0)
        units0[0]()  # x chunk-0 loads go out first
        for u in units0[1:]:
            u()
        for q in range(NQ):
            if q + 1 < TC:
                filler = ph1_units(q + 1)
                filler[0]()  # prefetch the chunk's x tiles at round start
                filler = filler[1:]
            else:
                filler = [tail_unit(tt) for tt in range(12)]
            fi = 0
            for h in range(HPC):
                attn_block(h, q)
                # spread filler units across the 8 heads
                take = (len(filler) - fi) // (HPC - h) if h < HPC else 0
                for _ in range(take):
                    filler[fi]()
                    fi += 1
            while fi < len(filler):
                filler[fi]()
                fi += 1
        flush_norm()
        for tt in range(12, NKT):
            tail_unit(tt)()

        o_pool.release()
        r_pool.release()
        e_pool.release()
        xt_pool.release()
        psum_bc.release()
        psum_aug.release()
        psum.release()
        persist.release()
        const.release()

    nc.finalize()
    return nc


_NC_CACHE = {}


def _get_nc(mm_dt=None):
    key = str(mm_dt or MM_DT)
    if key not in _NC_CACHE:
        _NC_CACHE[key] = build_nc(mm_dt)
    return _NC_CACHE[key]


def make_in_maps(x, Wqkv, bqkv, Wo, mm_dt=None):
    mdt = _np_of(mm_dt or MM_DT)
    x = np.asarray(x, dtype=np.float32)
    Wqkv = np.asarray(Wqkv, dtype=np.float32)
    bqkv = np.asarray(bqkv, dtype=np.float32)
    Wo = np.asarray(Wo, dtype=np.float32)

    w3 = Wqkv.reshape(C, 3, H, D)
    b3 = bqkv.reshape(3, H, D)
    wo4 = Wo.reshape(H, D, C)
    tri = np.triu(np.ones((128, 128), dtype=np.float32))

    in_maps = []
    for c in range(N_CORES):
        b, g = c // 2, c % 2
        hs = slice(g * HPC, (g + 1) * HPC)
        bq = b3[0, hs].reshape(HD)
        bk = b3[1, hs].reshape(HD)
        in_maps.append({
            "xT": np.ascontiguousarray(x[b].T).astype(mdt),
            "wq": np.ascontiguousarray(w3[:, 0, hs, :].reshape(C, HD)).astype(mdt),
            "wk": np.ascontiguousarray(w3[:, 1, hs, :].reshape(C, HD)).astype(mdt),
            "wv": np.ascontiguousarray(w3[:, 2, hs, :].reshape(C, HD)).astype(mdt),
            "wo": np.ascontiguousarray(wo4[hs].reshape(HD, C)).astype(mdt),
            "bqc": np.ascontiguousarray(bq.reshape(HD // 128, 128).T).astype(np.float32),
            "bkc": np.ascontiguousarray(bk.reshape(HD // 128, 128).T).astype(np.float32),
            "bv": b3[2, hs].reshape(1, HD).astype(mdt),
            "tri": tri.astype(mdt),
        })
    return in_maps


def run(x, Wqkv, bqkv, Wo, bo, mm_dt=None, **spmd_kwargs):
    nc = _get_nc(mm_dt)
    in_maps = make_in_maps(x, Wqkv, bqkv, Wo, mm_dt=mm_dt)
    res = run_bass_kernel_spmd(nc, in_maps, core_ids=list(range(N_CORES)),
                               **spmd_kwargs)
    bo = np.asarray(bo, dtype=np.float32)
    out = np.empty((B, T, C), dtype=np.float32)
    for b in range(B):
        out[b] = res.results[2 * b]["out"] + res.results[2 * b + 1]["out"] + bo
    return out, res


def kernel(x, Wqkv, bqkv, Wo, bo):
    out, _ = run(x, Wqkv, bqkv, Wo, bo)
    return out
